# revision 26
# baseline (speedup 1.0000x reference)
"""Trainium2 Bass kernel for nn_Bridge_57329223467265 (ragged repeat-interleave).

Reference computation (per batch row b of x [4, 2048, 512]):
    counts = argmax(x @ W + b_vec, -1)            # per-token repeat counts in [0,15]
    csum   = cumsum(counts)                        # inclusive
    out[p] = x[first j with csum[j] > p]  for p < csum[-1], else 0   # p in [0, 30720)

Sharding: 8 cores = 4 batch rows x 2 output halves. Each core receives its
batch row (full x row replicated to its pair), computes logits/argmax/cumsum
on-device, then materializes its 15360x512 output slice with one one-hot
selection matmul per 128-row chunk.

Selection matrices come straight from the cumsum with batched windowed
compares (source s covers slot g iff csum[s-1] <= g < csum[s]); the window
base for chunk k is w0[k] = #{j: csum[j] <= slot0_k}, quantized to 64 rows.
A second 64-row-shifted bf16 copy of x makes every 64-aligned window a
single [128, 512] rhs slice, so each chunk is exactly one matmul.

kernel(**inputs) takes full unsharded inputs and returns the full [4,30720,512]
output. Everything data-dependent is computed on the NeuronCores.
"""

import numpy as np

from concourse import bass, mybir, bacc, tile
from concourse import bass_utils
from concourse.masks import make_identity, make_upper_triangular

P = 128
S = 2048            # tokens per batch row
D = 512             # feature dim
NCLS = 16           # classes / max repeat-1
NB0 = 17            # phase0 x blocks in SBUF incl one zero pad block
NBT = 33            # total xb blocks: 17 phase0 + 16 phase1 (64-row shifted)
PH1 = NB0 * D       # element offset of phase1 region in xb
LMAX = S * (NCLS - 1)   # 30720
HALF = LMAX // 2        # 15360 rows per core
NCH = HALF // P         # 120 chunks of 128 output rows
GC = 8                  # chunks per batched sel-build group
NG = NCH // GC          # 15 groups
BIG = 1.0e9

F32 = mybir.dt.float32
BF16 = mybir.dt.bfloat16
F16 = mybir.dt.float16
FP8 = mybir.dt.float8e4
I32 = mybir.dt.int32
U32 = mybir.dt.uint32
OP = mybir.AluOpType
AX = mybir.AxisListType


def build():
    nc = bacc.Bacc("TRN2", target_bir_lowering=False, debug=False, num_devices=8)

    x_dram = nc.dram_tensor("x", [S, D], F32, kind="ExternalInput").ap()
    w_dram = nc.dram_tensor("w", [D, NCLS], F32, kind="ExternalInput").ap()
    b_dram = nc.dram_tensor("bvec", [1, NCLS], F32, kind="ExternalInput").ap()
    p0_dram = nc.dram_tensor("p0", [1, 1], F32, kind="ExternalInput").ap()
    out_dram = nc.dram_tensor("out", [HALF, D], BF16, kind="ExternalOutput").ap()

    with tile.TileContext(nc) as tc:
        _body(tc, x_dram, w_dram, b_dram, p0_dram, out_dram)

    nc.compile()
    return nc


def _body(tc, x_dram, w_dram, b_dram, p0_dram, out_dram):
    nc = tc.nc
    from contextlib import ExitStack

    with ExitStack() as ctx:
        const = ctx.enter_context(tc.tile_pool(name="const", bufs=1))
        work = ctx.enter_context(tc.tile_pool(name="work", bufs=1))
        pipe = ctx.enter_context(tc.tile_pool(name="pipe", bufs=4))

        # ---------------- static tiles ----------------
        ident = const.tile([P, P], F32, tag="ident")
        make_identity(nc, ident[:])
        ustr = const.tile([P, P], F32, tag="ustr")       # 1 where row<col
        make_upper_triangular(nc, ustr[:], 1.0, diag=False)
        ones1 = const.tile([1, P], F32, tag="ones1")
        nc.gpsimd.memset(ones1[:], 1.0)
        onescol = const.tile([P, 1], BF16, tag="onescol")
        nc.gpsimd.memset(onescol[:], 1.0)

        # ---------------- load inputs ----------------
        # four group tiles of 4 token-blocks each: few dma_start issues (the
        # sync sequencer pays ~0.6us per issue) but still fine-grained enough
        # that transposes start after the first group lands
        xsb = [const.tile([P, D], F32, tag=f"xsb{m}", name=f"xsb{m}") for m in range(16)]
        for q in range(4):
            nc.sync.dma_start(xsb[0][32 * q:32 * (q + 1), :],
                              x_dram[32 * q:32 * (q + 1), :])
        for m in range(1, 16):
            nc.sync.dma_start(xsb[m][:], x_dram[m * P:(m + 1) * P, :])

        w_sb = const.tile([P, 4 * NCLS], F32, tag="w_sb")
        for c in range(4):
            nc.sync.dma_start(w_sb[:, c * NCLS:(c + 1) * NCLS], w_dram[c * P:(c + 1) * P, :])
        b_sb = const.tile([1, NCLS], F32, tag="b_sb")
        nc.sync.dma_start(b_sb[:], b_dram[:])
        p0_sb = const.tile([1, 1], F32, tag="p0_sb")
        nc.sync.dma_start(p0_sb[:], p0_dram[:])

        # bf16 copy of x for the expand matmuls (~2^-9 relative; gate is 2e-2)
        xb = const.tile([P, NBT * D], BF16, tag="xb")
        for m in range(16):
            sl_ = slice(m * D, (m + 1) * D)
            if m % 2 == 1:
                nc.gpsimd.tensor_copy(xb[:, sl_], xsb[m][:])
            else:
                nc.vector.tensor_copy(xb[:, sl_], xsb[m][:])
        nc.gpsimd.memset(xb[:, 16 * D:17 * D], 0.0)

        # big iota consts on gpsimd, after the PE-critical ident/ustr
        # [s, (j, p)] = p for the batched sel compares
        iotaR = const.tile([P, 2 * GC * P], BF16, tag="iotaR")
        nc.gpsimd.iota(iotaR[:], pattern=[[0, 2 * GC], [1, P]], base=0,
                       channel_multiplier=0, allow_small_or_imprecise_dtypes=True)
        # [p, (m, k)] = 128k for the w0 count
        g0km = const.tile([P, 16 * NCH], F32, tag="g0km")
        nc.gpsimd.iota(g0km[:], pattern=[[0, 16], [P, NCH]], base=0,
                       channel_multiplier=0, allow_small_or_imprecise_dtypes=True)
        iota17f = const.tile([17, 1], F32, tag="iota17f")  # [a, 0] = a
        nc.gpsimd.iota(iota17f[:], pattern=[[0, 1]], base=0, channel_multiplier=1,
                       allow_small_or_imprecise_dtypes=True)
        # xb phase1 block m = x rows [64+128m, 192+128m): low half from phase0
        # block m partitions 64..127, high half from block m+1 partitions 0..63
        nc.sync.dma_start(xb[0:64, PH1:PH1 + 16 * D], xb[64:128, 0:16 * D])
        nc.sync.dma_start(xb[64:128, PH1:PH1 + 16 * D], xb[0:64, D:17 * D])

        # ---------------- xT + logits + counts ----------------
        with tc.tile_pool(name="psS", bufs=4, space="PSUM") as psS:
            # p0 broadcast + f16 slot-index tile first: they only need p0, so
            # they run long before the csum-dependent tail
            p0p = psS.tile([P, 1], F32, tag="sm", bufs=2)
            nc.tensor.matmul(p0p[:], lhsT=ones1[0:1, :], rhs=p0_sb[:],
                             start=True, stop=True)
            p0col = work.tile([P, 1], F32, tag="p0col")
            nc.vector.tensor_copy(p0col[:], p0p[:])
            p0m12 = work.tile([P, 1], F32, tag="p0m12")
            nc.vector.tensor_scalar(p0m12[:], p0col[:], -12.0, None, op0=OP.add)
            g0p0 = work.tile([P, 16 * NCH], F16, tag="g0p0")
            nc.vector.tensor_scalar(g0p0[:], g0km[:], p0m12[:, 0:1], None, op0=OP.add)

            # xT tiled by (c, t4) so each logit matmul only waits for its own
            # four transposes
            xT = [[const.tile([P, 4 * P], F32, tag=f"xT{c}_{t}", name=f"xT{c}_{t}")
                   for t in range(4)] for c in range(4)]
            for m in range(16):
                for c in range(4):
                    pt = psS.tile([P, P], F32, tag="tr", bufs=2)
                    nc.tensor.transpose(
                        pt[:], xsb[m][:, c * P:(c + 1) * P], ident[:]
                    )
                    nc.scalar.copy(xT[c][m // 4][:, (m % 4) * P:(m % 4 + 1) * P], pt[:])
            bcp = psS.tile([P, 1], F32, tag="sm", bufs=2)
            nc.tensor.transpose(bcp[0:16, 0:1], b_sb[:], ident[0:1, 0:1])
            bcol = work.tile([16, 1], F32, tag="bcol")
            nc.vector.tensor_copy(bcol[:], bcp[0:16, 0:1])

            cntf = const.tile([P, 16], F32, tag="cntf")
            for t4 in range(4):
                plT = psS.tile([16, 4 * P], F32, tag="lgT", bufs=2)
                for c in range(4):
                    nc.tensor.matmul(
                        plT[:], lhsT=w_sb[:, c * NCLS:(c + 1) * NCLS],
                        rhs=xT[c][t4][:],
                        start=(c == 0), stop=(c == 3),
                    )
                lgT = pipe.tile([16, 4 * P], F32, tag="lgT_sb")
                nc.vector.tensor_scalar(lgT[:], plT[:], bcol[:, 0:1], None, op0=OP.add)
                for u in range(4):
                    m = 4 * t4 + u
                    pb = psS.tile([P, NCLS], F32, tag="lg", bufs=2)
                    nc.tensor.transpose(pb[:, 0:16], lgT[:, u * P:(u + 1) * P],
                                        ident[0:16, 0:16])
                    lg = pipe.tile([P, NCLS], F32, tag="lg_sb")
                    nc.vector.tensor_copy(lg[:], pb[:, 0:16])
                    mx8 = pipe.tile([P, 8], F32, tag="mx8")
                    nc.vector.max(mx8[:], lg[:])
                    mi = pipe.tile([P, 8], U32, tag="mi")
                    nc.vector.max_index(mi[:], mx8[:], lg[:])
                    nc.vector.tensor_copy(cntf[:, m:m + 1], mi[:, 0:1])

            # counts [128,16] -> [16,128]
            ctp = psS.tile([P, P], F32, tag="tr", bufs=2)
            nc.tensor.transpose(ctp[0:16, :], cntf[:], ident[:])
            cT = work.tile([16, P], F32, tag="cT")
            nc.vector.tensor_copy(cT[:], ctp[0:16, :])

            # ---------------- csum (inclusive + exclusive) ----------------
            csl = work.tile([16, P], F32, tag="csl")
            nc.vector.tensor_tensor_scan(csl[:], cT[:], cT[:], 0.0, op0=OP.add, op1=OP.bypass)
            offp = psS.tile([P, 1], F32, tag="sm", bufs=2)
            nc.tensor.matmul(offp[0:16, :], lhsT=ustr[0:16, 0:16], rhs=csl[:, P - 1:P],
                             start=True, stop=True)
            # csum/ce written straight into the BIG-padded phase tiles
            cs0 = work.tile([17, P], F32, tag="cs0")
            nc.gpsimd.memset(cs0[:], BIG)
            ce0 = work.tile([17, P], F32, tag="ce0")
            nc.gpsimd.memset(ce0[:], BIG)
            csum = cs0[0:16, :]
            nc.vector.tensor_scalar(csum, csl[:], offp[0:16, 0:1], None, op0=OP.add)
            ce = ce0[0:16, :]
            nc.vector.tensor_tensor(ce, csum, cT[:], op=OP.subtract)

            # csum transposed: [p, m] = csum[token 128m+p], for the w0 count
            ctr = psS.tile([P, P], F32, tag="tr", bufs=2)
            nc.tensor.transpose(ctr[:, 0:16], csum, ident[0:16, 0:16])
            csumT16 = work.tile([P, 16], F16, tag="csumT16")
            nc.vector.tensor_copy(csumT16[:], ctr[:, 0:16])

            # w0[k] ~= #{j: csum[j] <= p0 + 128k} via one batched f16 compare
            # (approximation error is absorbed by the 64-row window slack; the
            # -12 bias keeps the quantized base at or below the true w0),
            # halving-tree sum over m, column-sum over p via matmul
            ind = work.tile([P, 16 * NCH], BF16, tag="ind")
            nc.vector.tensor_tensor(
                ind.rearrange("p (m k) -> p m k", m=16),
                g0p0.rearrange("p (m k) -> p m k", m=16),
                csumT16[:].rearrange("p (m o) -> p m o", o=1).broadcast_to([P, 16, NCH]),
                op=OP.is_ge)
            r8 = work.tile([P, 8 * NCH], BF16, tag="r8")
            nc.vector.tensor_tensor(r8[:], ind[:, 0:8 * NCH], ind[:, 8 * NCH:16 * NCH], op=OP.add)
            r4 = work.tile([P, 4 * NCH], BF16, tag="r4")
            nc.vector.tensor_tensor(r4[:], r8[:, 0:4 * NCH], r8[:, 4 * NCH:8 * NCH], op=OP.add)
            r2 = work.tile([P, 2 * NCH], BF16, tag="r2")
            nc.vector.tensor_tensor(r2[:], r4[:, 0:2 * NCH], r4[:, 2 * NCH:4 * NCH], op=OP.add)
            r1 = work.tile([P, NCH], BF16, tag="r1")
            nc.vector.tensor_tensor(r1[:], r2[:, 0:NCH], r2[:, NCH:2 * NCH], op=OP.add)
            w0p = psS.tile([1, NCH], F32, tag="sm", bufs=2)
            nc.tensor.matmul(w0p[:], lhsT=onescol[:], rhs=r1[:], start=True, stop=True)

            # B64 = w0 >> 6; half = B64 >> 1 (block); par = B64 & 1 (phase)
            w0i = work.tile([1, NCH], I32, tag="w0i")
            nc.vector.tensor_copy(w0i[:], w0p[:])
            b64i = work.tile([1, NCH], I32, tag="b64i")
            nc.vector.tensor_scalar(b64i[:], w0i[:], 6, None, op0=OP.arith_shift_right)

            # hp row [1, 2*NCH] = [half | par] as f32, broadcast to 17 partitions
            hpi = work.tile([1, 2 * NCH], I32, tag="hpi")
            nc.vector.tensor_scalar(hpi[0:1, 0:NCH], b64i[:], 1, None,
                                    op0=OP.arith_shift_right)
            nc.vector.tensor_scalar(hpi[0:1, NCH:2 * NCH], b64i[:], 1, None,
                                    op0=OP.bitwise_and)
            hp = work.tile([1, 2 * NCH], F32, tag="hp")
            nc.vector.tensor_copy(hp[:], hpi[:])
            hpp = psS.tile([17, 2 * NCH], F32, tag="lgT", bufs=2)
            nc.tensor.matmul(hpp[:], lhsT=ones1[0:1, 0:17], rhs=hp[:],
                             start=True, stop=True)
            hpb = work.tile([17, 2 * NCH], F32, tag="hpb")
            nc.vector.tensor_copy(hpb[:], hpp[:])
            # oh1 = (half == a) * par ; oh0 = (half == a) - oh1
            oh1 = work.tile([17, NCH], F32, tag="oh1")
            nc.vector.scalar_tensor_tensor(oh1[:], in0=hpb[:, 0:NCH],
                                           scalar=iota17f[:, 0:1],
                                           in1=hpb[:, NCH:2 * NCH],
                                           op0=OP.is_equal, op1=OP.mult)
            oh0 = work.tile([17, NCH], F32, tag="oh0")
            nc.vector.scalar_tensor_tensor(oh0[:], in0=hpb[:, 0:NCH],
                                           scalar=iota17f[:, 0:1], in1=oh1[:],
                                           op0=OP.is_equal, op1=OP.subtract)

            # 64-shifted phase tiles [17, 128]: row a = tokens [128a+64, ...)
            cs1 = work.tile([17, P], F32, tag="cs1")
            nc.gpsimd.memset(cs1[:], BIG)
            nc.vector.tensor_copy(cs1[0:16, 0:64], csum[:, 64:128])
            nc.sync.dma_start(cs1[0:15, 64:128], cs0[1:16, 0:64])
            ce1 = work.tile([17, P], F32, tag="ce1")
            nc.gpsimd.memset(ce1[:], BIG)
            nc.vector.tensor_copy(ce1[0:16, 0:64], ce[:, 64:128])
            nc.sync.dma_start(ce1[0:15, 64:128], ce0[1:16, 0:64])

            # gather per-chunk windowed csum columns, minus g0, interleaved into
            # CSB group blocks of 16: [CSE'(8) | CSW'(8)] per group
            CSB = const.tile([P, 2 * NCH], BF16, tag="CSB")
            g0v = g0km[:, 0:NCH]  # [p, k] = 128k (m=0 block of g0km)
            cep_ = psS.tile([P, NCH], F32, tag="lgT", bufs=2)
            nc.tensor.matmul(cep_[:], lhsT=ce0[:], rhs=oh0[:], start=True, stop=False)
            nc.tensor.matmul(cep_[:], lhsT=ce1[:], rhs=oh1[:], start=False, stop=True)
            nc.vector.scalar_tensor_tensor(
                CSB.rearrange("p (g t) -> p g t", t=2 * GC)[:, :, 0:GC],
                in0=cep_[:].rearrange("p (g c) -> p g c", c=GC),
                scalar=p0col[:, 0:1],
                in1=g0v.rearrange("p (g c) -> p g c", c=GC),
                op0=OP.subtract, op1=OP.subtract)
            csp_ = psS.tile([P, NCH], F32, tag="lgT", bufs=2)
            nc.tensor.matmul(csp_[:], lhsT=cs0[:], rhs=oh0[:], start=True, stop=False)
            nc.tensor.matmul(csp_[:], lhsT=cs1[:], rhs=oh1[:], start=False, stop=True)
            nc.vector.scalar_tensor_tensor(
                CSB.rearrange("p (g t) -> p g t", t=2 * GC)[:, :, GC:2 * GC],
                in0=csp_[:].rearrange("p (g c) -> p g c", c=GC),
                scalar=p0col[:, 0:1],
                in1=g0v.rearrange("p (g c) -> p g c", c=GC),
                op0=OP.subtract, op1=OP.subtract)

            # xb element offsets per chunk: off = (B64>>1)*512 + (B64&1)*PH1
            hsf = work.tile([1, NCH], F32, tag="hsf")
            nc.vector.tensor_scalar(hsf[:], hp[0:1, 0:NCH], float(D), None, op0=OP.mult)
            bofff = work.tile([1, NCH], F32, tag="bofff")
            nc.vector.scalar_tensor_tensor(bofff[:], in0=hp[0:1, NCH:2 * NCH],
                                           scalar=float(PH1),
                                           in1=hsf[:], op0=OP.mult, op1=OP.add)
            Boff = const.tile([1, NCH], I32, tag="Boff")
            nc.vector.tensor_copy(Boff[:], bofff[:])

        # ---------------- main expand loop ----------------
        with (
            tc.tile_pool(name="psO", bufs=8, space="PSUM") as psO,
            tc.tile_pool(name="ddp", bufs=2) as ddp,
            tc.tile_pool(name="selp", bufs=2) as selp,
            tc.tile_pool(name="outp", bufs=8) as outp,
        ):
            GRP = 16  # chunks per batched register load
            for k in range(NCH):
                if k % GRP == 0:
                    n = min(GRP, NCH - k)
                    _, vals = nc.values_load_multi_w_load_instructions(
                        Boff[0:1, k:k + n],
                        engines={mybir.EngineType.PE},
                        min_val=0, max_val=PH1 + 15 * D,
                        skip_runtime_bounds_check=True,
                    )
                voff = vals[k % GRP]

                if k % GC == 0:
                    g = k // GC
                    # DD[s, (j, p)] = (p >= CSB[s, 16g + j]); first 8 j-blocks
                    # threshold on CSE', last 8 on CSW'
                    dd = ddp.tile([P, 2 * GC * P], BF16, tag="dd")
                    nc.vector.tensor_tensor(
                        dd.rearrange("p (j q) -> p j q", q=P),
                        iotaR[:].rearrange("p (j q) -> p j q", q=P),
                        CSB[:, 2 * GC * g:2 * GC * (g + 1)]
                            .rearrange("p (j o) -> p j o", o=1)
                            .broadcast_to([P, 2 * GC, P]),
                        op=OP.is_ge)
                    # sel = (p >= CSE') - (p >= CSW') in {0, 1}
                    sel8 = selp.tile([P, GC * P], FP8, tag="sel8")
                    nc.vector.tensor_tensor(sel8[:], dd[:, 0:GC * P],
                                            dd[:, GC * P:2 * GC * P], op=OP.subtract)

                j = (k % GC) * P
                po = psO.tile([P, D], F32, tag="po")
                nc.tensor.matmul(po[:], lhsT=sel8[:, j:j + P],
                                 rhs=xb[:, bass.ds(voff, D)],
                                 start=True, stop=True)

                if k % 2 == 0:
                    ob = outp.tile([P, 2 * D], BF16, tag="ob")
                half_sl = slice((k % 2) * D, (k % 2) * D + D)
                if k % 4 == 3:
                    nc.vector.tensor_copy(ob[:, half_sl], po[:])
                else:
                    nc.scalar.copy(ob[:, half_sl], po[:])
                if k % 2 == 1:
                    nc.sync.dma_start(
                        out_dram[(k - 1) * P:(k + 1) * P, :].rearrange(
                            "(c p) d -> c p d", p=P).transpose([1, 0, 2]),
                        ob[:].rearrange("p (c d) -> p c d", c=2))


# ---------------------------------------------------------------------------
_BUILT = {}


def _get_built():
    if "k" not in _BUILT:
        _BUILT["k"] = build()
    return _BUILT["k"]


def make_in_maps(x, W, b):
    in_maps = []
    for core in range(8):
        bi, h = core // 2, core % 2
        in_maps.append({
            "x": np.ascontiguousarray(x[bi]).astype(np.float32),
            "w": np.ascontiguousarray(W).astype(np.float32),
            "bvec": np.ascontiguousarray(b).reshape(1, NCLS).astype(np.float32),
            "p0": np.array([[float(h * HALF)]], dtype=np.float32),
        })
    return in_maps


def assemble(outs):
    return np.stack(
        [np.concatenate([outs[2 * b], outs[2 * b + 1]], axis=0) for b in range(4)]
    ).astype(np.float32)


def kernel(x, W, b):
    nc = _get_built()
    res = bass_utils.run_bass_kernel_spmd(nc, make_in_maps(x, W, b),
                                          core_ids=list(range(8)))
    return assemble([res.results[c]["out"] for c in range(8)])


if __name__ == "__main__":
    nc = build()
    print("build OK")
